# revision 9
# baseline (speedup 1.0000x reference)
"""Trainium2 Bass kernel for nn_CorrelationLayer.

Math: out[b, 0, i, j] = sum_{c,y,x} feat1[b,c,y+i-2,x+j-2] * feat2[b,c,y,x]
with out-of-range feat1 reads contributing zero (16 shifted dot products
per batch over the (C, H, W) = (512, 4, 4) volume).

Strategy: pure data parallel over batch (8 cores x 512 batches), with a
hybrid split of each core's 512 batches across two compute pipelines:

* PE (tensor-engine) pipeline, 384 batches: per batch the 16x16 Gram
  matrix G = F1^T F2 (contracting C=512 on partitions, 4 accumulating
  128-chunk matmuls into PSUM; 96 batches packed per PSUM bank as
  3 col-groups x 32 free slots - psum quadrant 3 can't take matmul
  output). G goes PSUM -> SBUF (fp16, Act engine) -> DRAM -> SBUF,
  where the DRAM round trip re-lands G with batch on partitions.
  out[b,d] is then a strided diagonal sum over G: free offsets
  68*qy + 17*qx + (64*di + 16*dj + 68*y0 + 17*x0), one Act
  activation(Copy, accum_out) per displacement.
* DVE (vector-engine) pipeline, 128 batches: batch on partitions, fused
  scalar_tensor_tensor multiply+reduce per displacement (y-rows fused
  into a single instruction when di==0 or dj==0).

Inputs are cast to fp16 on the host (halves HBM traffic; PE runs at
full rate on 16-bit). Accumulation stays fp32 throughout.
"""

import sys

import numpy as np

sys.path.insert(0, "/opt/trn_rl_repo")

import concourse.bacc as bacc
import concourse.mybir as mybir
import concourse.tile as tile
from concourse import bass_utils

B, C, H, W = 4096, 512, 4, 4
NCORES = 8
BL = B // NCORES          # 512 batches per core
F = C * H * W             # 8192 elements per batch
S = H * W                 # 16 spatial positions

NPE = 384                 # PE-pipeline batches per core
NDV = BL - NPE            # DVE-pipeline batches per core (one 128-tile)
PER_BANK = 96             # PE batches per psum bank (3 col groups x 32)
NBANK = NPE // PER_BANK   # 4
HALF = 192                # PE batches per double-buffered input half
CHUNKS = 4                # C / 128

_cached_nc = {}


def _disp_geom(i, j):
    di, dj = i - 2, j - 2
    y0, y1 = max(0, -di), min(H - 1, H - 1 - di)
    x0, x1 = max(0, -dj), min(W - 1, W - 1 - dj)
    return di, dj, y0, y1, x0, x1


def _emit_dve(nc, t1, t2, prod, acc):
    """Fused multiply+reduce on DVE for all 16 displacements of one
    128-batch tile (batch on partitions)."""
    a1 = t1.rearrange("p (c y x) -> p c y x", y=H, x=W)
    a2 = t2.rearrange("p (c y x) -> p c y x", y=H, x=W)
    ap = prod.rearrange("p (c y x) -> p c y x", y=H, x=W)
    a14 = t1.rearrange("p (cy x) -> p cy x", x=W)
    a24 = t2.rearrange("p (cy x) -> p cy x", x=W)
    ap4 = prod.rearrange("p (cy x) -> p cy x", x=W)
    a1r = t1.rearrange("p (c yx) -> p c yx", yx=S)
    a2r = t2.rearrange("p (c yx) -> p c yx", yx=S)
    apr = prod.rearrange("p (c yx) -> p c yx", yx=S)

    mult = mybir.AluOpType.mult
    for i in range(4):
        for j in range(4):
            di, dj, y0, y1, x0, x1 = _disp_geom(i, j)
            base = (i * 4 + j) * 4
            if dj == 0:
                w1 = a1r[:, :, (y0 + di) * W:(y1 + di) * W + W]
                w2 = a2r[:, :, y0 * W:y0 * W + (y1 - y0 + 1) * W]
                po = apr[:, :, y0 * W:y0 * W + (y1 - y0 + 1) * W]
                nc.vector.scalar_tensor_tensor(
                    out=po, in0=w1, scalar=1.0, in1=w2, op0=mult, op1=mult,
                    accum_out=acc[:, base:base + 1])
            elif di == 0:
                w1 = a14[:, :, x0 + dj:x1 + 1 + dj]
                w2 = a24[:, :, x0:x1 + 1]
                po = ap4[:, :, x0:x1 + 1]
                nc.vector.scalar_tensor_tensor(
                    out=po, in0=w1, scalar=1.0, in1=w2, op0=mult, op1=mult,
                    accum_out=acc[:, base:base + 1])
            else:
                for y in range(y0, y1 + 1):
                    w1 = a1[:, :, y + di, x0 + dj:x1 + 1 + dj]
                    w2 = a2[:, :, y, x0:x1 + 1]
                    po = ap[:, :, y, x0:x1 + 1]
                    nc.vector.scalar_tensor_tensor(
                        out=po, in0=w1, scalar=1.0, in1=w2, op0=mult, op1=mult,
                        accum_out=acc[:, base + (y - y0):base + (y - y0) + 1])


def _emit_body(nc, tc, fpe1, fpe2, fel1, fel2, gbuf, outd):
    fp16 = mybir.dt.float16
    fp32 = mybir.dt.float32
    with (
        tc.tile_pool(name="pei", bufs=2) as peip,
        tc.tile_pool(name="gs", bufs=2) as gsp,
        tc.tile_pool(name="gt", bufs=2) as gtp,
        tc.tile_pool(name="fa", bufs=2) as fap,
        tc.tile_pool(name="eli", bufs=2) as elip,
        tc.tile_pool(name="sc", bufs=1) as scp,
        tc.tile_pool(name="ac", bufs=2) as acp,
        tc.tile_pool(name="ps", bufs=2, space="PSUM") as psp,
    ):

        # ---------------- DVE pipeline: 128 batches ----------------
        prod = scp.tile([128, F], fp16, tag="prod", name="prod")
        t1 = elip.tile([128, F], fp16, tag="t1", name="t1")
        t2 = elip.tile([128, F], fp16, tag="t2", name="t2")
        nc.sync.dma_start(out=t1[:], in_=fel1[:, :])
        nc.sync.dma_start(out=t2[:], in_=fel2[:, :])
        acc = acp.tile([128, 64], fp32, tag="acc", name="acc")
        fin = acp.tile([128, 16], fp32, tag="fin", name="fin")
        nc.vector.memset(acc[:], 0.0)
        _emit_dve(nc, t1, t2, prod, acc)
        nc.vector.tensor_reduce(
            out=fin[:],
            in_=acc.rearrange("p (d y) -> p d y", y=4),
            axis=mybir.AxisListType.X,
            op=mybir.AluOpType.add,
        )
        nc.sync.dma_start(out=outd[NPE:NPE + NDV, :], in_=fin[:])

        # ---------------- PE pipeline: 384 batches ----------------
        gb = gbuf.rearrange("r (p j q) -> r p j q", p=16, j=32)
        gbr = gbuf.rearrange("r (p j q) -> r j p q", p=16, j=32)
        for h in range(2):           # halves of 192 batches
            pf1 = peip.tile([128, CHUNKS * HALF * S], fp16, tag="pf1", name="pf1")
            pf2 = peip.tile([128, CHUNKS * HALF * S], fp16, tag="pf2", name="pf2")
            for k in range(CHUNKS):
                r0 = (h * CHUNKS + k) * 128
                nc.sync.dma_start(out=pf1[:, k * HALF * S:(k + 1) * HALF * S],
                                  in_=fpe1[r0:r0 + 128, :])
                nc.sync.dma_start(out=pf2[:, k * HALF * S:(k + 1) * HALF * S],
                                  in_=fpe2[r0:r0 + 128, :])
            for bank in range(2 * h, 2 * h + 2):
                ps = psp.tile([128, 512], fp32, tag="ps", name="ps")
                for j in range(32):
                    for g in range(3):
                        s = bank * PER_BANK + 32 * g + j
                        r = s - h * HALF
                        for k in range(CHUNKS):
                            off = k * HALF * S + r * S
                            nc.tensor.matmul(
                                out=ps[32 * g:32 * g + 16, 16 * j:16 * j + 16],
                                lhsT=pf1[:, off:off + S],
                                rhs=pf2[:, off:off + S],
                                start=(k == 0),
                                stop=(k == CHUNKS - 1),
                            )
                gs = gsp.tile([128, 512], fp16, tag="gs", name="gs")
                nc.scalar.copy(out=gs[:], in_=ps[:])
                for g in range(3):
                    nc.sync.dma_start(out=gb[bank * 3 + g],
                                      in_=gs[32 * g:32 * g + 16, :])
        # readback + fold (per bank: 96 batches on partitions)
        for bank in range(NBANK):
            gt = gtp.tile([PER_BANK, 256], fp16, tag="gt", name="gt")
            for g in range(3):
                nc.sync.dma_start(out=gt[32 * g:32 * g + 32, :],
                                  in_=gbr[bank * 3 + g])
            facc = fap.tile([PER_BANK, 16], fp32, tag="facc", name="facc")
            fscr = fap.tile([PER_BANK, 16], fp16, tag="fscr", name="fscr")
            for i in range(4):
                for j in range(4):
                    di, dj, y0, y1, x0, x1 = _disp_geom(i, j)
                    ny, nx = y1 - y0 + 1, x1 - x0 + 1
                    base = 68 * y0 + 17 * x0 + 64 * di + 16 * dj
                    # strided diagonal view: [p, ny, nx] strides (68, 17)
                    anchor = gt[:, base:base + 1]
                    src = tile.bass.AP(
                        anchor.tensor, anchor.offset,
                        [list(anchor.ap[0]), [68, ny], [17, nx]])
                    d = i * 4 + j
                    nc.scalar.activation(
                        out=fscr[:, :ny * nx].rearrange("p (a b) -> p a b", b=nx),
                        in_=src,
                        func=mybir.ActivationFunctionType.Copy,
                        accum_out=facc[:, d:d + 1],
                    )
            nc.sync.dma_start(
                out=outd[bank * PER_BANK:(bank + 1) * PER_BANK, :], in_=facc[:])


def _build(reps: int = 1):
    nc = bacc.Bacc("TRN2", target_bir_lowering=False, debug=False)
    fp16 = mybir.dt.float16
    fpe1 = nc.dram_tensor("fpe1", [2 * CHUNKS * 128, HALF * S], fp16,
                          kind="ExternalInput").ap()
    fpe2 = nc.dram_tensor("fpe2", [2 * CHUNKS * 128, HALF * S], fp16,
                          kind="ExternalInput").ap()
    fel1 = nc.dram_tensor("fel1", [NDV, F], fp16, kind="ExternalInput").ap()
    fel2 = nc.dram_tensor("fel2", [NDV, F], fp16, kind="ExternalInput").ap()
    gbuf = nc.dram_tensor("gbuf", [NBANK * 3, 16 * 32 * 16], fp16,
                          kind="ExternalOutput").ap()
    outd = nc.dram_tensor("out", [BL, 16], mybir.dt.float32,
                          kind="ExternalOutput").ap()

    with tile.TileContext(nc) as tc:
        if reps == 1:
            _emit_body(nc, tc, fpe1, fpe2, fel1, fel2, gbuf, outd)
        else:
            with tc.For_i(0, reps, 1):
                _emit_body(nc, tc, fpe1, fpe2, fel1, fel2, gbuf, outd)

    nc.compile()
    return nc


def _get_nc(reps: int = 1):
    if reps not in _cached_nc:
        _cached_nc[reps] = _build(reps)
    return _cached_nc[reps]


def _prep_core(f1c, f2c):
    """Host-side shard prep for one core's 512 batches (fp32 [512, C, H, W])."""
    ins = {}
    for name, a in (("1", f1c), ("2", f2c)):
        pe = a[:NPE].astype(np.float16)          # [384, 512, 4, 4]
        # -> [half(2), chunk(4), c(128), b(192), s(16)] -> rows [(h k) c]
        pe = pe.reshape(2, HALF, CHUNKS, 128, S).transpose(0, 2, 3, 1, 4)
        ins[f"fpe{name}"] = np.ascontiguousarray(
            pe.reshape(2 * CHUNKS * 128, HALF * S))
        ins[f"fel{name}"] = np.ascontiguousarray(
            a[NPE:].astype(np.float16).reshape(NDV, F))
    return ins


def kernel(feat1, feat2):
    f1 = np.asarray(feat1, dtype=np.float32).reshape(B, C, H, W)
    f2 = np.asarray(feat2, dtype=np.float32).reshape(B, C, H, W)
    nc = _get_nc()
    in_maps = [
        _prep_core(f1[k * BL:(k + 1) * BL], f2[k * BL:(k + 1) * BL])
        for k in range(NCORES)
    ]
    res = bass_utils.run_bass_kernel_spmd(nc, in_maps, list(range(NCORES)))
    out = np.concatenate([res.results[k]["out"] for k in range(NCORES)], axis=0)
    return out.reshape(B, 1, H, W)
